# revision 1
# baseline (speedup 1.0000x reference)
"""BD3LM block-diffusion decoder layer on 8 trn2 NeuronCores.

Sharding: core = 2*b + g  (b = batch 0..3, g = head-group 0..1, 8 heads each).
Each core: QKV projections for its batch/head-group, sparse BD3LM attention
(only ~80 of 256 score tiles per head), O-projection against its Wo row-slice.
Host: sums the two group partials per batch and adds the (bv @ Wo + bo)
correction (softmax rows sum to 1, so the v-bias contributes exactly bv @ Wo).

Layouts on device (per core):
  qT/kT  [d_head_group=512, T=2048]  stored [128, 4, 2048]  (d on partitions)
  v      [T, 512] stored [128, 16, 8*65] with a per-head ones column -> the
         ctx matmul accumulates softmax denominators for free (row 64).
  scores computed transposed [k_tile=128, q_span] so softmax reduction is a
         PE matmul instead of a partition reduction; exp on ACT without
         max-subtraction (scores are ~N(0,1), bias-free overflow impossible);
         only 3 distinct 128x128 binary mask tiles (strict/incl/diag).
"""

import numpy as np

import concourse.bass as bass
import concourse.mybir as mybir
import concourse.tile as tile
from concourse import bacc
from concourse.bass_utils import run_bass_kernel_spmd

F32 = mybir.dt.float32
F32R = mybir.dt.float32r
Act = mybir.ActivationFunctionType

B, T, D = 4, 2048, 1024
H, HD = 16, 64
L = T // 2           # 1024, length of each of [xt | x0]
BS = 4               # block size
G = 2                # head groups (cores per batch)
DG = D // G          # 512 channels per group
HG = H // G          # 8 heads per core
P = 128
NT = L // P          # 8 key/query tiles per half
SLAB = 256           # projection t-slab width
KC = D // P          # 8 contraction chunks
DT4 = DG // P        # 4 output-partition tiles for qT/kT

# matmul dtype per family: float32 (exact, 4 cyc/row) or float32r (~2.7e-4
# end-to-end, 1 cyc/row at N>=256). f32r measured ~2.2x faster end-to-end.
PROJ_DT = F32R
ATTN_DT = F32R
OPROJ_DT = F32R
BCAST_DT = F32R

REPEAT = 1  # loop whole computation inside the NEFF (timing experiments only)
DBG = False

_CACHE = {}


def _chunks512(a0, a1):
    """Split [a0, a1) at multiples of 512 (PSUM bank boundaries)."""
    out = []
    while a0 < a1:
        b1 = min(a1, (a0 // 512 + 1) * 512)
        out.append((a0, b1))
        a0 = b1
    return out


def _mm(ap, dt):
    return ap.bitcast(dt) if dt != F32 else ap


def _build():
    import concourse.tile_utils as tile_utils

    tile_utils.max_sbuf_usage = 204 * 1024  # trn2 has 208KB/partition usable

    nc = bacc.Bacc("TRN2", target_bir_lowering=False, debug=False, num_devices=8)
    dbg = {}
    if DBG:
        for nm, shp in (
            ("dbg_qT", [P, DT4, T]),
            ("dbg_kT", [P, DT4, T]),
            ("dbg_v", [P, T // P, HG * (HD + 1)]),
            ("dbg_ctxT", [P, DT4, T]),
            ("dbg_nd", [16, HD + 1, L]),
            ("dbg_at", [P, L]),
        ):
            dbg[nm] = nc.dram_tensor(nm, shp, F32, kind="ExternalOutput").ap()

    xT = nc.dram_tensor("xT", [D, T], F32, kind="ExternalInput").ap()
    wq = nc.dram_tensor("wq", [D, DG], F32, kind="ExternalInput").ap()
    wk = nc.dram_tensor("wk", [D, DG], F32, kind="ExternalInput").ap()
    wv = nc.dram_tensor("wv", [D, DG], F32, kind="ExternalInput").ap()
    wo = nc.dram_tensor("wo", [DG, D], F32, kind="ExternalInput").ap()
    bqs = nc.dram_tensor("bqs", [DG], F32, kind="ExternalInput").ap()
    bks = nc.dram_tensor("bks", [DG], F32, kind="ExternalInput").ap()
    msk = nc.dram_tensor("msk", [3, P, P], F32, kind="ExternalInput").ap()
    out = nc.dram_tensor("out", [T, D], F32, kind="ExternalOutput").ap()

    views = dict(
        xT_v=xT.rearrange("(kc p) t -> p kc t", p=P),    # [128, 8, 2048]
        wq_v=wq.rearrange("(kc p) m -> p kc m", p=P),    # [128, 8, 512]
        wk_v=wk.rearrange("(kc p) m -> p kc m", p=P),
        wv_v=wv.rearrange("(kc p) m -> p kc m", p=P),
        wo_v=wo.rearrange("(cc p) n -> p cc n", p=P),    # [128, 4, 1024]
        msk=msk,
        out=out,
    )

    with tile.TileContext(nc) as tc:
        with tc.tile_pool(name="persist", bufs=1) as pers:
            st = dict(
                qT_sb=pers.tile([P, DT4, T], F32, name="qT_sb"),
                kT_sb=pers.tile([P, DT4, T], F32, name="kT_sb"),
                v_sb=pers.tile([P, T // P, HG * (HD + 1)], F32, name="v_sb"),
                bq_sb=pers.tile([P, DT4], F32, name="bq_sb"),
                bk_sb=pers.tile([P, DT4], F32, name="bk_sb"),
            )
            nc.sync.dma_start(st["bq_sb"], bqs.rearrange("(c p) -> p c", p=P))
            nc.sync.dma_start(st["bk_sb"], bks.rearrange("(c p) -> p c", p=P))
            # ones columns for the softmax denominators
            ones_c = pers.tile([P, 1], F32, name="ones_c")
            nc.vector.memset(ones_c, 1.0)
            ones_v = st["v_sb"].rearrange("p t (h c) -> p (t h) c", c=HD + 1)[
                :, :, HD : HD + 1
            ]
            if ATTN_DT == F32:
                nc.vector.memset(ones_v, 1.0)
            else:
                nc.vector.tensor_copy(
                    _mm(ones_v, ATTN_DT),
                    ones_c[:, 0:1, None].to_broadcast(tuple(ones_v.shape)),
                )
            st["ones_c"] = ones_c

            for _rep in range(REPEAT):
                _phases(nc, tc, dbg, st, views)

    nc.compile()
    return nc


def _phases(nc, tc, dbg, st, views):
    qT_sb, kT_sb, v_sb = st["qT_sb"], st["kT_sb"], st["v_sb"]
    xT_v, wo_v, msk, out = views["xT_v"], views["wo_v"], views["msk"], views["out"]

    # ---------------- Phase A: QKV projections (one x stream) ----------------
    with (
        tc.tile_pool(name="wpool", bufs=1) as wpool,
        tc.tile_pool(name="xpool", bufs=3) as xpool,
        tc.tile_pool(name="ppsum", bufs=4, space="PSUM") as ppsum,
        tc.tile_pool(name="vpsum", bufs=4, space="PSUM") as vpsum,
    ):
        wq_sb = wpool.tile([P, KC, DG], F32, name="wq_sb")
        wk_sb = wpool.tile([P, KC, DG], F32, name="wk_sb")
        wv_sb = wpool.tile([P, KC, DG], F32, name="wv_sb")
        x_tiles = []
        for s in range(T // 512):
            x_sb = xpool.tile([P, KC, 512], F32, tag="x", name=f"x{s}")
            if s < 2:  # prefetch depth 2; later slabs DMA'd in the loop
                nc.sync.dma_start(
                    _mm(x_sb, PROJ_DT),
                    _mm(xT_v[:, :, 512 * s : 512 * (s + 1)], PROJ_DT),
                )
            x_tiles.append(x_sb)
        # wq split per column-tile: the d4=0 matmuls only wait on 0.5MB of wq
        for d4 in range(DT4):
            nc.sync.dma_start(
                _mm(wq_sb[:, :, P * d4 : P * (d4 + 1)], PROJ_DT),
                _mm(views["wq_v"][:, :, P * d4 : P * (d4 + 1)], PROJ_DT),
            )
        nc.sync.dma_start(_mm(wk_sb, PROJ_DT), _mm(views["wk_v"], PROJ_DT))
        nc.sync.dma_start(_mm(wv_sb, PROJ_DT), _mm(views["wv_v"], PROJ_DT))
        for s in range(T // 512):
            x_sb = x_tiles[s]
            if s >= 2:
                nc.sync.dma_start(
                    _mm(x_sb, PROJ_DT),
                    _mm(xT_v[:, :, 512 * s : 512 * (s + 1)], PROJ_DT),
                )
            for w_sb, b_key, dst, scale in (
                (wq_sb, "bq_sb", qT_sb, HD ** -0.5),
                (wk_sb, "bk_sb", kT_sb, 1.0),
            ):
                for d4 in range(DT4):
                    ps = ppsum.tile([P, 512], F32, tag="pp", name=f"pp{s}_{d4}")
                    for kc in range(KC):
                        nc.tensor.matmul(
                            ps,
                            _mm(w_sb[:, kc, P * d4 : P * (d4 + 1)], PROJ_DT),
                            _mm(x_sb[:, kc, :], PROJ_DT),
                            start=(kc == 0),
                            stop=(kc == KC - 1),
                        )
                    nc.scalar.activation(
                        _mm(dst[:, d4, 512 * s : 512 * (s + 1)], ATTN_DT),
                        ps,
                        Act.Identity,
                        bias=st[b_key][:, d4 : d4 + 1],
                        scale=scale,
                    )
            for t2 in range(4):
                tt = 4 * s + t2
                ps = vpsum.tile([P, DG], F32, tag="ppv", name=f"ppv{tt}")
                for kc in range(KC):
                    nc.tensor.matmul(
                        ps,
                        _mm(x_sb[:, kc, P * t2 : P * (t2 + 1)], PROJ_DT),
                        _mm(wv_sb[:, kc, :], PROJ_DT),
                        start=(kc == 0),
                        stop=(kc == KC - 1),
                    )
                nc.vector.tensor_copy(
                    _mm(
                        v_sb[:, tt].rearrange("p (h c) -> p h c", c=HD + 1)[:, :, :HD],
                        ATTN_DT,
                    ),
                    ps.rearrange("p (h c) -> p h c", c=HD),
                )

    # ---------------- Phase B: sparse attention ----------------
    with (
        tc.tile_pool(name="apool", bufs=1) as apool,
        tc.tile_pool(name="tmppool", bufs=2) as tmppool,
    ):
        ctxT_sb = apool.tile([P, DT4, T], F32, name="ctxT_sb")
        wo_sb = apool.tile([P, DT4, D], F32, name="wo_sb")
        nc.sync.dma_start(_mm(wo_sb, OPROJ_DT), _mm(wo_v, OPROJ_DT))
        m_strict = apool.tile([P, P], F32, name="m_strict")
        m_incl = apool.tile([P, P], F32, name="m_incl")
        m_diag = apool.tile([P, P], F32, name="m_diag")
        nc.sync.dma_start(m_strict, msk[0])
        nc.sync.dma_start(m_incl, msk[1])
        nc.sync.dma_start(m_diag, msk[2])
        ones_t = apool.tile([P, HD], F32, name="ones_t")  # row 64: K=1 bcast lhsT
        if BCAST_DT == F32:
            nc.vector.memset(ones_t, 1.0)
        else:
            nc.vector.tensor_copy(
                _mm(ones_t, BCAST_DT),
                st["ones_c"][:, 0:1].to_broadcast((P, HD)),
            )

        from contextlib import ExitStack as _ES

        _es = _ES()
        atpool = _es.enter_context(tc.tile_pool(name="atpool", bufs=8))
        spsum = _es.enter_context(tc.tile_pool(name="spsum", bufs=3, space="PSUM"))
        cpsum = _es.enter_context(tc.tile_pool(name="cpsum", bufs=2, space="PSUM"))
        for h in range(HG):
            c, p0 = h // 2, HD * (h % 2)
            qh = qT_sb[p0 : p0 + HD, c, :]   # [64, 2048]
            kh = kT_sb[p0 : p0 + HD, c, :]
            for half in range(2):
                ctx = cpsum.tile([HD + 1, L], F32, tag="ctx", name=f"ctx{h}_{half}")
                mask = m_strict if half == 0 else m_incl
                for j in range(NT):
                    kv = kh[:, L + P * j : L + P * (j + 1)]                  # [64, 128]
                    vj = v_sb[:, NT + j, (HD + 1) * h : (HD + 1) * (h + 1)]  # [128, 65]
                    for a0, a1 in _chunks512(P * j, L):
                        n = a1 - a0
                        sc = spsum.tile(
                            [P, 512], F32, tag="sc", name=f"sc{h}_{j}_{half}_{a0}"
                        )[:, :n]
                        nc.tensor.matmul(
                            sc,
                            _mm(kv, ATTN_DT),
                            _mm(qh[:, L * half + a0 : L * half + a1], ATTN_DT),
                            start=True,
                            stop=True,
                        )
                        at = atpool.tile(
                            [P, 512], F32, tag="at", name=f"at{h}_{j}_{half}_{a0}"
                        )[:, :n]
                        nc.scalar.activation(_mm(at, ATTN_DT), sc, Act.Exp)
                        if a0 == P * j:
                            nc.vector.tensor_mul(
                                _mm(at[:, :P], ATTN_DT), at[:, :P], mask
                            )
                        if DBG and h == 0 and j == 0 and half == 1:
                            nc.sync.dma_start(dbg["dbg_at"][:, a0:a1], at)
                        # x0 half: stop on the last j touching this bank
                        last = half == 1 and (
                            (a1 <= 512 and j == 3) or (a0 >= 512 and j == NT - 1)
                        )
                        nc.tensor.matmul(
                            ctx[:, a0:a1],
                            _mm(vj, ATTN_DT),
                            _mm(at, ATTN_DT),
                            start=(j == 0),
                            stop=last,
                        )
                if half == 0:
                    # xt-xt block-diagonal tiles
                    for i in range(NT):
                        scd = spsum.tile(
                            [P, 512], F32, tag="sc", name=f"scd{h}_{i}"
                        )[:, :P]
                        nc.tensor.matmul(
                            scd,
                            _mm(kh[:, P * i : P * (i + 1)], ATTN_DT),
                            _mm(qh[:, P * i : P * (i + 1)], ATTN_DT),
                            start=True,
                            stop=True,
                        )
                        atd = atpool.tile(
                            [P, 512], F32, tag="at", name=f"atd{h}_{i}"
                        )[:, :P]
                        nc.scalar.activation(_mm(atd, ATTN_DT), scd, Act.Exp)
                        nc.vector.tensor_mul(_mm(atd, ATTN_DT), atd, m_diag)
                        nc.tensor.matmul(
                            ctx[:, P * i : P * (i + 1)],
                            _mm(v_sb[:, i, (HD + 1) * h : (HD + 1) * (h + 1)], ATTN_DT),
                            _mm(atd, ATTN_DT),
                            start=False,
                            stop=(i == 3 or i == NT - 1),
                        )
                if DBG:
                    ndc = tmppool.tile(
                        [HD + 1, L], F32, tag="ndc", name=f"ndc{h}_{half}"
                    )
                    nc.scalar.activation(ndc, ctx, Act.Copy)
                    nc.sync.dma_start(dbg["dbg_nd"][2 * h + half], ndc)
                # normalize: ctxT = ctx[:64] * (1 / denom), denom = row 64
                recip = tmppool.tile([P, L], F32, tag="recip", name=f"rc{h}_{half}")
                with nc.allow_low_precision(reason="deliberate f32r rounding"):
                    nc.vector.reciprocal(
                        _mm(recip[HD : HD + 1, :], BCAST_DT),
                        ctx[HD : HD + 1, :],
                    )
                rb = tmppool.tile([HD, L], F32, tag="rb", bufs=3, name=f"rb{h}_{half}")
                # PE broadcast: ones[1,64].T @ recip[1,n] -> [64, n]
                for c0 in range(0, L, 512):
                    bc = spsum.tile(
                        [P, 512], F32, tag="bc", bufs=1, name=f"bc{h}_{half}_{c0}"
                    )[:HD, :]
                    nc.tensor.matmul(
                        bc,
                        _mm(ones_t[HD : HD + 1, :], BCAST_DT),
                        _mm(recip[HD : HD + 1, c0 : c0 + 512], BCAST_DT),
                        start=True,
                        stop=True,
                    )
                    nc.vector.tensor_copy(rb[:, c0 : c0 + 512], bc)
                if h % 2 == 0:
                    nc.vector.tensor_mul(
                        _mm(ctxT_sb[:HD, c, L * half : L * (half + 1)], OPROJ_DT),
                        ctx[:HD, :],
                        rb,
                    )
                else:
                    cs = tmppool.tile([HD, L], F32, tag="cs", bufs=3, name=f"cs{h}_{half}")
                    nc.vector.tensor_mul(_mm(cs, OPROJ_DT), ctx[:HD, :], rb)
                    nc.sync.dma_start(
                        _mm(ctxT_sb[HD : 2 * HD, c, L * half : L * (half + 1)], OPROJ_DT),
                        _mm(cs, OPROJ_DT),
                    )

        if DBG:
            nc.sync.dma_start(dbg["dbg_qT"], qT_sb)
            nc.sync.dma_start(dbg["dbg_kT"], kT_sb)
            nc.sync.dma_start(dbg["dbg_v"], v_sb)
            nc.sync.dma_start(dbg["dbg_ctxT"], ctxT_sb)

        _es.close()

        # ---------------- Phase C: O-projection ----------------
        with tc.tile_pool(name="opsum", bufs=6, space="PSUM") as opsum:
            for tt in range(T // P):
                for nk in range(2):
                    ops = opsum.tile([P, 512], F32, tag="op", name=f"op{tt}_{nk}")
                    for cc in range(DT4):
                        nc.tensor.matmul(
                            ops,
                            _mm(ctxT_sb[:, cc, P * tt : P * (tt + 1)], OPROJ_DT),
                            _mm(wo_sb[:, cc, 512 * nk : 512 * (nk + 1)], OPROJ_DT),
                            start=(cc == 0),
                            stop=(cc == DT4 - 1),
                        )
                    osb = tmppool.tile([P, 512], F32, tag="osb", bufs=6, name=f"osb{tt}_{nk}")
                    nc.vector.tensor_copy(osb, ops)
                    nc.sync.dma_start(
                        out[P * tt : P * (tt + 1), 512 * nk : 512 * (nk + 1)], osb
                    )


def _masks():
    q = np.arange(P)[None, :] // BS
    k = np.arange(P)[:, None] // BS
    m = np.zeros((3, P, P), np.float32)
    m[0] = (q > k).astype(np.float32)    # strict (xt q vs x0 k, same tile)
    m[1] = (q >= k).astype(np.float32)   # incl (x0 q vs x0 k, same tile)
    m[2] = (q == k).astype(np.float32)   # diag (xt q vs xt k, same tile)
    return m


def kernel(x, Wq, bq, Wk, bk, Wv, bv, Wo, bo, block_size=4, **_):
    x = np.asarray(x, np.float32)
    Wq, bq = np.asarray(Wq, np.float32), np.asarray(bq, np.float32)
    Wk, bk = np.asarray(Wk, np.float32), np.asarray(bk, np.float32)
    Wv, bv = np.asarray(Wv, np.float32), np.asarray(bv, np.float32)
    Wo, bo = np.asarray(Wo, np.float32), np.asarray(bo, np.float32)

    if "nc" not in _CACHE:
        _CACHE["nc"] = _build()
    nc = _CACHE["nc"]

    masks = _masks()
    scale = HD ** -0.5
    in_maps = []
    for core in range(8):
        b, g = core // 2, core % 2
        cols = slice(DG * g, DG * (g + 1))
        in_maps.append(
            {
                "xT": np.ascontiguousarray(x[b].T),
                "wq": np.ascontiguousarray(Wq[:, cols]),
                "wk": np.ascontiguousarray(Wk[:, cols]),
                "wv": np.ascontiguousarray(Wv[:, cols]),
                "wo": np.ascontiguousarray(Wo[cols, :]),
                "bqs": np.ascontiguousarray(bq[cols]) * np.float32(scale),
                "bks": np.ascontiguousarray(bk[cols]),
                "msk": masks,
            }
        )

    _CACHE["last_in_maps"] = in_maps
    last_err = None
    for _attempt in range(6):
        try:
            res = run_bass_kernel_spmd(nc, in_maps, core_ids=list(range(8)), trace=False)
            break
        except Exception as e:  # transient NRT device flakes
            last_err = e
            msg = str(e)
            if "UNRECOVERABLE" not in msg and "UNAVAILABLE" not in msg:
                raise
            import time as _time

            import jax as _jax

            _time.sleep(5 * (_attempt + 1))
            try:
                _jax.clear_backends()
            except Exception:
                pass
    else:
        raise last_err

    corr = (bv @ Wo + bo).astype(np.float32)  # softmax rows sum to 1
    out = np.empty((B, T, D), np.float32)
    for b in range(B):
        out[b] = res.results[2 * b]["out"] + res.results[2 * b + 1]["out"] + corr
    return out


if __name__ == "__main__":
    rng = np.random.default_rng(0)
    inputs = {
        "x": rng.standard_normal((B, T, D)).astype(np.float32),
        "Wq": (rng.standard_normal((D, D)) / 32).astype(np.float32),
        "bq": np.zeros(D, np.float32),
        "Wk": (rng.standard_normal((D, D)) / 32).astype(np.float32),
        "bk": np.zeros(D, np.float32),
        "Wv": (rng.standard_normal((D, D)) / 32).astype(np.float32),
        "bv": np.zeros(D, np.float32),
        "Wo": (rng.standard_normal((D, D)) / 32).astype(np.float32),
        "bo": np.zeros(D, np.float32),
    }
    o = kernel(**inputs)
    print("ran", o.shape, o.dtype, float(np.abs(o).max()))



# revision 12
# speedup vs baseline: 1.6202x; 1.6202x over previous
"""BD3LM block-diffusion decoder layer on 8 trn2 NeuronCores — v2 (bf16).

Sharding: core = 2*b + g  (b = batch 0..3, g = head-group 0..1, 8 heads each).
All matmuls bf16 (inputs converted on host; hd^-0.5 folded into Wq; bk dropped
— a per-query constant score shift is softmax-invariant). f32 PSUM accum.

Key structures vs v1:
  - Block masks applied ADDITIVELY on the PE: scores += A^T B where A[r,k] =
    [k//4 == r] (rank-32 indicator) and B[r,q] carries NEG on disallowed
    (block_r, q) pairs. One extra 128-col matmul per boundary tile; exp then
    covers a whole (head, half, j) span in ONE activation instruction.
  - Per-head-pair QKV projection, emission-interleaved with the previous
    pair's attention so the PE never waits on ACT's exp stream.
  - Softmax denominators via ones-columns in v (ctx row 64); reciprocal rows
    are broadcast across 64 partitions by a DRAM round-trip DMA; normalize is
    fused into the ctx eviction multiply on DVE.
  - Phase C (O-projection) interleaved into the last head's attention tail.
"""

import numpy as np
import ml_dtypes

import concourse.bass as bass
import concourse.mybir as mybir
import concourse.tile as tile
from concourse import bacc
from concourse.bass_utils import run_bass_kernel_spmd

F32 = mybir.dt.float32
BF16 = mybir.dt.bfloat16
Act = mybir.ActivationFunctionType

B, T, D = 4, 2048, 1024
H, HD = 16, 64
L = T // 2           # 1024
BS = 4               # block size
P = 128
NT = L // P          # 8 key/query tiles per half
PAIRS = 4            # head-pairs per core
KC = D // P          # 8 contraction chunks
NEG = -60.0          # additive mask value

REPEAT = 1  # loop whole computation inside the NEFF (timing experiments only)
DBG = False
INTERLEAVE = True

_CACHE = {}


def _chunks512(a0, a1):
    """Split [a0, a1) at multiples of 512 (PSUM bank boundaries)."""
    res = []
    while a0 < a1:
        b1 = min(a1, (a0 // 512 + 1) * 512)
        res.append((a0, b1))
        a0 = b1
    return res


def _mask_arrays():
    """A [32,128] indicator; B patterns [32,128] (strict/incl); Bd8 [32,1024]."""
    A = np.zeros((32, P), np.float32)
    A[np.arange(P) // BS, np.arange(P)] = 1.0
    r = np.arange(32)[:, None]
    qb = (np.arange(P) // BS)[None, :]
    Bs = np.where(r >= qb, NEG, 0.0).astype(np.float32)   # xt q vs x0 k: allow r < qb
    Bi = np.where(r > qb, NEG, 0.0).astype(np.float32)    # x0 q vs x0 k: allow r <= qb
    Bd = np.where(r != qb, NEG, 0.0).astype(np.float32)   # xt q vs xt k: allow r == qb
    Bd8 = np.tile(Bd, (1, NT))                             # [32, 1024]
    bf = ml_dtypes.bfloat16
    return A.astype(bf), Bs.astype(bf), Bi.astype(bf), Bd8.astype(bf)


def _build():
    import concourse.tile_utils as tile_utils

    tile_utils.max_sbuf_usage = 204 * 1024

    nc = bacc.Bacc("TRN2", target_bir_lowering=False, debug=False, num_devices=8)

    xT = nc.dram_tensor("xT", [D, T], BF16, kind="ExternalInput").ap()
    wq = nc.dram_tensor("wq", [D, D // 2], BF16, kind="ExternalInput").ap()
    wk = nc.dram_tensor("wk", [D, D // 2], BF16, kind="ExternalInput").ap()
    wv = nc.dram_tensor("wv", [D, D // 2], BF16, kind="ExternalInput").ap()
    wo = nc.dram_tensor("wo", [D // 2, D], BF16, kind="ExternalInput").ap()
    bqs = nc.dram_tensor("bqs", [D // 2], F32, kind="ExternalInput").ap()
    out = nc.dram_tensor("out", [T, D], F32, kind="ExternalOutput").ap()
    scratch = nc.dram_tensor(
        "scratch", [16, L], F32, kind="ExternalOutput" if DBG else "Internal"
    ).ap()
    dbg = {}
    if DBG:
        for nm, shp, dt in (
            ("dbg_qT", [P, PAIRS, T], F32),
            ("dbg_kT", [P, PAIRS, T], F32),
            ("dbg_v", [P, PAIRS, T // P, 2 * (HD + 1)], F32),
            ("dbg_ctxT", [P, PAIRS, T], F32),
        ):
            dbg[nm] = nc.dram_tensor(nm, shp, dt, kind="ExternalOutput").ap()

    mA_np, mBs_np, mBi_np, mBd8_np = _mask_arrays()
    mA_d = nc.inline_tensor(mA_np, "mA_d").ap()
    mBs_d = nc.inline_tensor(mBs_np, "mBs_d").ap()
    mBi_d = nc.inline_tensor(mBi_np, "mBi_d").ap()
    mBd_d = nc.inline_tensor(mBd8_np, "mBd_d").ap()

    views = dict(
        xT_v=xT.rearrange("(kc p) t -> p kc t", p=P),      # [128, 8, 2048]
        wq_v=wq.rearrange("(kc p) m -> p kc m", p=P),      # [128, 8, 512]
        wk_v=wk.rearrange("(kc p) m -> p kc m", p=P),
        wv_v=wv.rearrange("(kc p) m -> p kc m", p=P),
        wo_v=wo.rearrange("(cc p) n -> p cc n", p=P),      # [128, 4, 1024]
        bqs_v=bqs.rearrange("(c p) -> p c", p=P),          # [128, 4]
        mA=mA_d, mBs=mBs_d, mBi=mBi_d, mBd=mBd_d,
        out=out, scratch=scratch, dbg=dbg,
    )

    with tile.TileContext(nc) as tc:
        with tc.tile_pool(name="persist", bufs=1) as pers:
            st = dict(
                x_sb=pers.tile([P, KC, T], BF16, name="x_sb"),
                qT=pers.tile([P, PAIRS, T], BF16, name="qT"),
                kT=pers.tile([P, PAIRS, T], BF16, name="kT"),
                v_sb=pers.tile([P, PAIRS, T // P, 2 * (HD + 1)], BF16, name="v_sb"),
                ctxT=pers.tile([P, PAIRS, T], BF16, name="ctxT"),
                wq_sb=pers.tile([P, KC, D // 2], BF16, name="wq_sb"),
                wk_sb=pers.tile([P, KC, D // 2], BF16, name="wk_sb"),
                wv_sb=pers.tile([P, KC, D // 2], BF16, name="wv_sb"),
                wo_sb=pers.tile([P, PAIRS, D], BF16, name="wo_sb"),
                bq_sb=pers.tile([P, PAIRS], F32, name="bq_sb"),
                mA_sb=pers.tile([32, P], BF16, name="mA_sb"),
                mBs_sb=pers.tile([32, P], BF16, name="mBs_sb"),
                mBi_sb=pers.tile([32, P], BF16, name="mBi_sb"),
                mBd_sb=pers.tile([32, NT * P], BF16, name="mBd_sb"),
            )
            # ones columns (64 and 129) for the softmax denominators
            vv = st["v_sb"]
            nc.vector.memset(vv[:, :, :, HD : HD + 1], 1.0)
            nc.vector.memset(vv[:, :, :, 2 * HD + 1 : 2 * HD + 2], 1.0)
            nc.sync.dma_start(st["mA_sb"], views["mA"])
            nc.sync.dma_start(st["mBs_sb"], views["mBs"])
            nc.sync.dma_start(st["mBi_sb"], views["mBi"])
            nc.sync.dma_start(st["mBd_sb"], views["mBd"])
            nc.sync.dma_start(st["bq_sb"], views["bqs_v"])

            for _rep in range(REPEAT):
                _phases(nc, tc, st, views)

    nc.compile()
    return nc


def _phases(nc, tc, st, views):
    x_sb, qT, kT, v_sb, ctxT = (
        st["x_sb"], st["qT"], st["kT"], st["v_sb"], st["ctxT"]
    )
    scratch, out = views["scratch"], views["out"]

    # input DMAs, ordered by first use: pair-0 weights + x slab 0 first
    cols0 = slice(0, P)
    nc.sync.dma_start(st["wq_sb"][:, :, cols0], views["wq_v"][:, :, cols0])
    nc.sync.dma_start(st["wk_sb"][:, :, cols0], views["wk_v"][:, :, cols0])
    nc.sync.dma_start(st["wv_sb"][:, :, cols0], views["wv_v"][:, :, cols0])
    for s in range(4):
        nc.sync.dma_start(
            x_sb[:, :, 512 * s : 512 * (s + 1)],
            views["xT_v"][:, :, 512 * s : 512 * (s + 1)],
        )
    for p in range(1, PAIRS):
        cols = slice(P * p, P * (p + 1))
        nc.sync.dma_start(st["wq_sb"][:, :, cols], views["wq_v"][:, :, cols])
        nc.sync.dma_start(st["wk_sb"][:, :, cols], views["wk_v"][:, :, cols])
        nc.sync.dma_start(st["wv_sb"][:, :, cols], views["wv_v"][:, :, cols])
    nc.sync.dma_start(st["wo_sb"], views["wo_v"])

    from contextlib import ExitStack

    es = ExitStack()
    atp = es.enter_context(tc.tile_pool(name="atp", bufs=10))
    ctxs_p = es.enter_context(tc.tile_pool(name="ctxs_p", bufs=3))
    rbp = es.enter_context(tc.tile_pool(name="rbp", bufs=3))
    rcp = es.enter_context(tc.tile_pool(name="rcp", bufs=2))
    osbp = es.enter_context(tc.tile_pool(name="osbp", bufs=4))
    spsum = es.enter_context(tc.tile_pool(name="spsum", bufs=2, space="PSUM"))
    cpsum = es.enter_context(tc.tile_pool(name="cpsum", bufs=1, space="PSUM"))

    uid = [0]

    def nid():
        uid[0] += 1
        return uid[0]

    # ---------------- emission units ----------------

    def proj_units(apsum, p):
        """QKV projection for head-pair p as a list of small closures."""
        units = []
        cols = slice(P * p, P * (p + 1))

        def qk_chunk(dst, w_sb, tchunk, is_q):
            def go():
                ps = apsum.tile([P, 512], F32, tag="pp", name=f"pp{nid()}")
                for kc in range(KC):
                    nc.tensor.matmul(
                        ps,
                        w_sb[:, kc, cols],
                        x_sb[:, kc, 512 * tchunk : 512 * (tchunk + 1)],
                        start=(kc == 0),
                        stop=(kc == KC - 1),
                    )
                dv = dst[:, p, 512 * tchunk : 512 * (tchunk + 1)]
                if is_q:
                    nc.vector.tensor_scalar_add(dv, ps, st["bq_sb"][:, p : p + 1])
                else:
                    nc.vector.tensor_copy(dv, ps)
            return go

        def v_tile(tt):
            def go():
                ps = apsum.tile([P, P], F32, tag="pp", name=f"ppv{nid()}")
                xt = x_sb[:, :, P * tt : P * (tt + 1)]
                for kc in range(KC):
                    nc.tensor.matmul(
                        ps,
                        xt[:, kc, :],
                        st["wv_sb"][:, kc, cols],
                        start=(kc == 0),
                        stop=(kc == KC - 1),
                    )
                dv = v_sb[:, p, tt].rearrange("p (h c) -> p h c", c=HD + 1)[:, :, :HD]
                nc.vector.tensor_copy(dv, ps.rearrange("p (h c) -> p h c", c=HD))
            return go

        for tchunk in range(4):
            units.append(qk_chunk(qT, st["wq_sb"], tchunk, True))
            units.append(qk_chunk(kT, st["wk_sb"], tchunk, False))
            for t2 in range(4):
                units.append(v_tile(4 * tchunk + t2))
        return units

    def attn_head(h, ticks):
        """Emit attention for local head h; ticks[half]() at interleave slots."""
        p, r0 = h // 2, HD * (h % 2)
        qh = qT[r0 : r0 + HD, p, :]
        kh = kT[r0 : r0 + HD, p, :]
        vcol = (HD + 1) * (h % 2)

        for half in range(2):
            tick = ticks[half]
            ats = []
            # scores for all j (keys = x0 tiles)
            for j in range(NT):
                span = L - P * j
                kv = kh[:, L + P * j : L + P * (j + 1)]
                sc = spsum.tile([P, 1024], F32, tag="sc", name=f"sc{nid()}")
                q0 = L * half + P * j
                for c0 in range(0, span, 512):
                    c1 = min(span, c0 + 512)
                    nc.tensor.matmul(
                        sc[:, c0:c1], kv, qh[:, q0 + c0 : q0 + c1],
                        start=True, stop=True,
                    )
                mB = st["mBs_sb"] if half == 0 else st["mBi_sb"]
                nc.tensor.matmul(sc[:, 0:P], st["mA_sb"], mB, start=False, stop=True)
                at = atp.tile([P, 1024], BF16, tag="at", name=f"at{nid()}")[:, :span]
                nc.scalar.activation(at, sc[:, :span], Act.Exp)
                ats.append(at)
                if j % 2 == 1:
                    tick()
            # xt-xt block-diagonal scores (half 0 only)
            if half == 0:
                scd = spsum.tile([P, 1024], F32, tag="sc", name=f"scd{nid()}")
                for i in range(NT):
                    # start=True clears has_written for the WHOLE bank: only
                    # the first write per 512-col bank may set it
                    nc.tensor.matmul(
                        scd[:, P * i : P * (i + 1)],
                        kh[:, P * i : P * (i + 1)],
                        qh[:, P * i : P * (i + 1)],
                        start=(i % 4 == 0), stop=True,
                    )
                for c in range(2):
                    nc.tensor.matmul(
                        scd[:, 512 * c : 512 * (c + 1)],
                        st["mA_sb"],
                        st["mBd_sb"][:, 512 * c : 512 * (c + 1)],
                        start=False, stop=True,
                    )
                atd = atp.tile([P, 1024], BF16, tag="at", name=f"atd{nid()}")
                nc.scalar.activation(atd, scd, Act.Exp)
                tick()

            # ctx accumulation over j (+ diag for half 0); chunks at absolute
            # 512-boundaries of the ctx tile (PSUM banks)
            ctx = cpsum.tile([HD + 1, L], F32, tag="ctx", name=f"ctx{nid()}")
            for j in range(NT):
                vj = v_sb[:, p, NT + j, vcol : vcol + HD + 1]
                for a0, a1 in _chunks512(P * j, L):
                    c0 = a0 - P * j
                    last = half == 1 and (
                        (a1 <= 512 and j == 3) or (a0 >= 512 and j == NT - 1)
                    )
                    nc.tensor.matmul(
                        ctx[:, a0:a1], vj, ats[j][:, c0 : c0 + (a1 - a0)],
                        start=(j == 0), stop=last,
                    )
                if j % 2 == 1:
                    tick()
            if half == 0:
                for i in range(NT):
                    vi = v_sb[:, p, i, vcol : vcol + HD + 1]
                    nc.tensor.matmul(
                        ctx[:, P * i : P * (i + 1)], vi, atd[:, P * i : P * (i + 1)],
                        start=False, stop=(i == 3 or i == NT - 1),
                    )
                tick()

            # evict ctx, 1/denom, DRAM-trip partition broadcast, normalize.
            # Split the eviction per bank so ctx's PSUM frees earlier (WAR).
            ctx_s = ctxs_p.tile([HD + 1, L], F32, tag="cs", name=f"cs{nid()}")
            nc.vector.tensor_copy(ctx_s[:, 0:512], ctx[:, 0:512])
            nc.vector.tensor_copy(ctx_s[:, 512:L], ctx[:, 512:L])
            rc = rcp.tile([1, L], F32, tag="rc", name=f"rc{nid()}")
            nc.vector.reciprocal(rc, ctx_s[HD : HD + 1, :])
            row = 2 * h + half
            nc.sync.dma_start(scratch[row : row + 1, :], rc)
            rb = rbp.tile([HD, L], F32, tag="rb", name=f"rb{nid()}")
            nc.sync.dma_start(rb, scratch[row : row + 1, :].to_broadcast((HD, L)))
            nc.vector.tensor_mul(
                ctxT[r0 : r0 + HD, p, L * half : L * (half + 1)],
                ctx_s[:HD, :],
                rb,
            )
            tick()

    def oproj_units(opsum, half, dve_evict):
        """O-projection units for output rows of one half."""
        units = []

        def o_unit(tt, nk):
            def go():
                ops = opsum.tile([P, 512], F32, tag="op", name=f"op{nid()}")
                for cc in range(PAIRS):
                    nc.tensor.matmul(
                        ops,
                        ctxT[:, cc, P * tt : P * (tt + 1)],
                        st["wo_sb"][:, cc, 512 * nk : 512 * (nk + 1)],
                        start=(cc == 0),
                        stop=(cc == PAIRS - 1),
                    )
                osb = osbp.tile([P, 512], F32, tag="osb", name=f"osb{nid()}")
                if dve_evict:
                    nc.vector.tensor_copy(osb, ops)
                else:
                    nc.scalar.activation(osb, ops, Act.Copy)
                nc.sync.dma_start(
                    out[P * tt : P * (tt + 1), 512 * nk : 512 * (nk + 1)], osb
                )
            return go

        for tt in range(NT * half, NT * (half + 1)):
            for nk in range(2):
                units.append(o_unit(tt, nk))
        return units

    # ---------------- schedule ----------------

    def make_tick(pending):
        state = [0]

        def tick():
            state[0] += 1
            if pending and state[0] % 2 == 0:
                pending.pop(0)()
        return tick

    noop = lambda: None

    if INTERLEAVE:
        with tc.tile_pool(name="apsum", bufs=2, space="PSUM") as apsum:
            for u in proj_units(apsum, 0):
                u()
            for p in range(3):
                pending = proj_units(apsum, p + 1)
                tick = make_tick(pending)
                attn_head(2 * p, (tick, tick))
                attn_head(2 * p + 1, (tick, tick))
                for u in pending:
                    u()

        with tc.tile_pool(name="opsum", bufs=2, space="PSUM") as opsum:
            attn_head(6, (noop, noop))
            pending = oproj_units(opsum, 0, dve_evict=True)  # xt rows
            tick = make_tick(pending)
            attn_head(7, (noop, tick))
            for u in pending:
                u()
            for u in oproj_units(opsum, 1, dve_evict=False):  # x0 rows
                u()
    else:
        with tc.tile_pool(name="apsum", bufs=2, space="PSUM") as apsum:
            for p in range(4):
                for u in proj_units(apsum, p):
                    u()
        for h in range(8):
            attn_head(h, (noop, noop))
        with tc.tile_pool(name="opsum", bufs=2, space="PSUM") as opsum:
            for half in range(2):
                for u in oproj_units(opsum, half, dve_evict=False):
                    u()

    if views["dbg"]:
        dbg = views["dbg"]
        with tc.tile_pool(name="dbgp", bufs=2) as dbgp:
            for nm, src in (
                ("dbg_qT", qT), ("dbg_kT", kT), ("dbg_ctxT", ctxT)
            ):
                for pp in range(PAIRS):
                    t32 = dbgp.tile([P, T], F32, tag="d32", name=f"d{nid()}")
                    nc.vector.tensor_copy(t32, src[:, pp, :])
                    nc.sync.dma_start(dbg[nm][:, pp, :], t32)
            for pp in range(PAIRS):
                t32 = dbgp.tile([P, T // P * 2 * (HD + 1)], F32, tag="d32", name=f"dv{nid()}")
                tv = t32.rearrange("p (t c) -> p t c", c=2 * (HD + 1))
                nc.vector.tensor_copy(tv, v_sb[:, pp])
                nc.sync.dma_start(dbg["dbg_v"][:, pp], tv)

    es.close()


def kernel(x, Wq, bq, Wk, bk, Wv, bv, Wo, bo, block_size=4, **_):
    assert int(block_size) == BS
    bf = ml_dtypes.bfloat16
    x = np.asarray(x, np.float32)
    Wq, bq = np.asarray(Wq, np.float32), np.asarray(bq, np.float32)
    Wk = np.asarray(Wk, np.float32)
    Wv, bv = np.asarray(Wv, np.float32), np.asarray(bv, np.float32)
    Wo, bo = np.asarray(Wo, np.float32), np.asarray(bo, np.float32)

    if "nc" not in _CACHE:
        _CACHE["nc"] = _build()
    nc = _CACHE["nc"]

    scale = np.float32(HD ** -0.5)
    in_maps = []
    for core in range(8):
        b, g = core // 2, core % 2
        cols = slice(D // 2 * g, D // 2 * (g + 1))
        in_maps.append(
            {
                "xT": np.ascontiguousarray(x[b].T).astype(bf),
                "wq": np.ascontiguousarray(Wq[:, cols] * scale).astype(bf),
                "wk": np.ascontiguousarray(Wk[:, cols]).astype(bf),
                "wv": np.ascontiguousarray(Wv[:, cols]).astype(bf),
                "wo": np.ascontiguousarray(Wo[cols, :]).astype(bf),
                "bqs": np.ascontiguousarray(bq[cols]) * scale,
            }
        )

    _CACHE["last_in_maps"] = in_maps
    last_err = None
    for _attempt in range(6):
        try:
            res = run_bass_kernel_spmd(nc, in_maps, core_ids=list(range(8)), trace=False)
            break
        except Exception as e:  # transient NRT device flakes
            last_err = e
            msg = str(e)
            if "UNRECOVERABLE" not in msg and "UNAVAILABLE" not in msg:
                raise
            import time as _time

            import jax as _jax

            _time.sleep(5 * (_attempt + 1))
            try:
                _jax.clear_backends()
            except Exception:
                pass
    else:
        raise last_err

    corr = (bv @ Wo + bo).astype(np.float32)  # softmax rows sum to 1
    outv = np.empty((B, T, D), np.float32)
    for b in range(B):
        outv[b] = res.results[2 * b]["out"] + res.results[2 * b + 1]["out"] + corr
    return outv


if __name__ == "__main__":
    rng = np.random.default_rng(0)
    inputs = {
        "x": rng.standard_normal((B, T, D)).astype(np.float32),
        "Wq": (rng.standard_normal((D, D)) / 32).astype(np.float32),
        "bq": np.zeros(D, np.float32),
        "Wk": (rng.standard_normal((D, D)) / 32).astype(np.float32),
        "bk": np.zeros(D, np.float32),
        "Wv": (rng.standard_normal((D, D)) / 32).astype(np.float32),
        "bv": np.zeros(D, np.float32),
        "Wo": (rng.standard_normal((D, D)) / 32).astype(np.float32),
        "bo": np.zeros(D, np.float32),
    }
    o = kernel(**inputs)
    print("ran", o.shape, o.dtype, float(np.abs(o).max()))
